# revision 2
# baseline (speedup 1.0000x reference)
"""TT-embedding lookup kernel for 8 trn2 NeuronCores.

Strategy: the expanded embedding table (1e6 x 128 f32) is materialized
densely across the 8 cores, each core covering 1/8 of the (i0, i1) pair
range, then the host performs the final index->row gather (the unshard
step). The tiny first contraction AB[pair] = a_{i0} @ b_{i1} (164 MFLOP
total, 0.4% of the work) is folded into host-side input prep; the device
runs the heavy stage -- for every pair block of 32 pairs:

  OUT[(pair,q0)=128, (i2,q2)=800] = ABt_q1.T @ C2_q1   per q1 (K=r2=32)

with the four q1 matmuls packed into distinct PE row-groups
(tile_position) and distinct PSUM banks. PSUM is drained with two
N=800 bank-spanning copies per 4-bank tile, split across the Vector and
Scalar engines, casting f32 -> f16. Output tiles [128, 3200] f16 are
written with one 819 KB line-rate DMA per pair block.

Per-core HBM write traffic: 41 * 819.2 KB = 33.6 MB (f16), ~94 us at the
358 GB/s HBM-per-core limit -- the design target.

Shapes hardcoded from the problem spec:
  P=(100,100,100), Q=(4,4,8), R=(1,32,32,1), N=1<<20.
"""

import numpy as np

P0, P1, P2 = 100, 100, 100
Q0, Q1, Q2 = 4, 4, 8
R1, R2 = 32, 32
NCORES = 8

# i0 ranges per core: first 4 cores take 13 i0 values, last 4 take 12 (=100).
NI0 = [13, 13, 13, 13, 12, 12, 12, 12]
I0_BASE = np.cumsum([0] + NI0)[:-1]  # [0,13,26,39,52,64,76,88]
NI0_PAD = 13                      # padded i0 count per core (uniform program)
NPAIR = P1 * NI0_PAD              # 1300
NPAIR_PAD = 1312                  # 41 blocks of 32 pairs
NPB = NPAIR_PAD // 32             # 41 pair-blocks
WCOLS = P2 * Q2                   # 800 output cols (i2, q2)
ACOLS = NPAIR_PAD * Q0            # 5248 abt cols (pair, q0)

_cache = {}


def _build_program():
    from concourse import bacc
    import concourse.mybir as mybir
    from concourse.tile import TileContext

    f32 = mybir.dt.float32
    f16 = mybir.dt.float16

    nc = bacc.Bacc("TRN2", target_bir_lowering=False, debug=False,
                   num_devices=NCORES)

    abt = nc.dram_tensor("abt", [128, ACOLS], f16, kind="ExternalInput")
    c2r = nc.dram_tensor("c2r", [128, WCOLS], f16, kind="ExternalInput")
    out = nc.dram_tensor("out", [NPB, 128, Q1 * WCOLS], f16,
                         kind="ExternalOutput")

    with TileContext(nc) as tc:
        with tc.tile_pool(name="const", bufs=1) as cpool, \
             tc.tile_pool(name="ps", bufs=2, space="PSUM") as pspool, \
             tc.tile_pool(name="osb", bufs=3) as opool:

            abt_sb = cpool.tile([128, ACOLS], f16, tag="abt")
            c2_sb = cpool.tile([128, WCOLS], f16, tag="c2")
            nc.sync.dma_start(out=abt_sb[:], in_=abt[:])
            nc.sync.dma_start(out=c2_sb[:], in_=c2r[:])

            for pb in range(NPB):
                osb = opool.tile([128, Q1 * WCOLS], f16, tag="osb")
                for half in range(2):
                    # one 4-bank PSUM tile holds two q1 outputs, each at a
                    # 1024-col (2-bank) stride so every matmul stays inside
                    # a single bank: q1a at [0:800], q1b at [1024:1824].
                    ps = pspool.tile([128, 2048], f32, tag="ps")
                    for j in range(2):
                        q1 = 2 * half + j
                        base = j * 1024
                        lhsT = abt_sb[32 * q1:32 * (q1 + 1),
                                      pb * 128:(pb + 1) * 128]
                        nc.tensor.matmul(
                            ps[:, base:base + 512], lhsT,
                            c2_sb[32 * q1:32 * (q1 + 1), 0:512],
                            start=True, stop=True,
                            tile_position=(32 * q1, 0))
                        nc.tensor.matmul(
                            ps[:, base + 512:base + 800], lhsT,
                            c2_sb[32 * q1:32 * (q1 + 1), 512:800],
                            start=True, stop=True,
                            tile_position=(32 * q1, 0))
                    qa, qb = 2 * half, 2 * half + 1
                    nc.vector.tensor_copy(
                        osb[:, qa * WCOLS:(qa + 1) * WCOLS], ps[:, 0:800])
                    nc.scalar.copy(
                        osb[:, qb * WCOLS:(qb + 1) * WCOLS],
                        ps[:, 1024:1824])
                nc.sync.dma_start(out=out[pb], in_=osb[:])
    nc.finalize()
    return nc


def _host_inputs(core0, core1, core2):
    """Shared prep + per-core input maps.

    AB[i0, i1, q0, (q1 r2)] = a_{i0} @ b_{i1} -- 164 MFLOP on host.
    Per core: abt[(q1 r2)=128, (i1, i0loc, q0)] f16, c2r[(q1 r2), (i2 q2)].
    """
    a0 = core0.reshape(P0 * Q0, R1)                       # [(i0 q0), r1]
    b = core1.reshape(P1, R1, Q1 * R2).transpose(1, 0, 2).reshape(
        R1, P1 * Q1 * R2)                                 # [r1, (i1 qr)]
    ab = (a0 @ b).reshape(P0, Q0, P1, Q1 * R2)            # [i0, q0, i1, qr]
    ab = ab.transpose(0, 2, 1, 3)                         # [i0, i1, q0, qr]

    c2 = core2.reshape(P2, R2, Q2).transpose(1, 0, 2).reshape(R2, P2 * Q2)
    c2r = np.ascontiguousarray(np.tile(c2, (Q1, 1)), np.float16)

    in_maps = []
    for k in range(NCORES):
        base, ni0 = I0_BASE[k], NI0[k]
        i0s = np.arange(base, base + ni0)
        i0s = np.concatenate([i0s, np.repeat(i0s[-1:], NI0_PAD - ni0)])
        abk = ab[i0s]                                     # [13, i1, q0, qr]
        abk = abk.transpose(3, 1, 0, 2).reshape(128, P1 * NI0_PAD * Q0)
        abt = np.zeros((128, ACOLS), np.float16)
        abt[:, :P1 * NI0_PAD * Q0] = abk
        in_maps.append({"abt": abt, "c2r": c2r})
    return in_maps


def run_device(core0, core1, core2, trace=False):
    from concourse.bass_utils import run_bass_kernel_spmd
    if "nc" not in _cache:
        _cache["nc"] = _build_program()
    nc = _cache["nc"]
    in_maps = _host_inputs(core0, core1, core2)
    res = run_bass_kernel_spmd(nc, in_maps, core_ids=list(range(NCORES)),
                               trace=trace)
    return res


def _reassemble(raw):
    # raw: [pb, (ps,q0), (q1,i2,q2)] -> value table [pair*100 + i2, 128] f16
    vt = raw.reshape(NPB, 32, Q0, Q1, P2, Q2)
    vt = vt.transpose(0, 1, 4, 2, 3, 5)          # [pb, ps, i2, q0, q1, q2]
    return np.ascontiguousarray(vt).reshape(NPAIR_PAD * P2, Q0 * Q1 * Q2)


def kernel(core0, core1, core2, indices):
    core0 = np.asarray(core0, np.float32)
    core1 = np.asarray(core1, np.float32)
    core2 = np.asarray(core2, np.float32)
    idx = np.asarray(indices)

    res = run_device(core0, core1, core2)
    vts = [_reassemble(r["out"]) for r in res.results]

    i0 = (idx // (P1 * P2)).astype(np.int64)
    i1 = ((idx // P2) % P1).astype(np.int64)
    i2 = (idx % P2).astype(np.int64)
    core_of = np.zeros(P0, np.int64)
    for k in range(NCORES):
        core_of[I0_BASE[k]:I0_BASE[k] + NI0[k]] = k
    ck = core_of[i0]

    out = np.empty((idx.shape[0], Q0 * Q1 * Q2), np.float32)
    for k in range(NCORES):
        m = ck == k
        rows = (i1[m] * NI0_PAD + (i0[m] - I0_BASE[k])) * P2 + i2[m]
        out[m] = vts[k][rows]
    return out


# revision 3
# speedup vs baseline: 1.3254x; 1.3254x over previous
"""TT-embedding lookup kernel for 8 trn2 NeuronCores.

Strategy: the expanded embedding table (1e6 rows x 128 cols) is
materialized densely across the 8 cores -- core k owns the pair range
p = i0*100+i1 in [1250k, 1250(k+1)) -- and the host performs the final
index->row gather (the unshard step). The tiny first contraction
AB[pair] = a_{i0} @ b_{i1} (164 MFLOP total, ~0.4% of the FLOPs) is
folded into host-side input prep; the device runs the heavy stage:
for every block of 32 pairs,

  OUT[(pair,q0)=128, (i2,q2)=800] = ABt_q1.T @ C2_q1   per q1 (K=r2=32)

as fp16 matmuls (1 col/cycle on the PE vs 1/4 for f32) packed into
distinct PE row-groups (tile_position) and distinct PSUM banks. Each
4-bank PSUM tile is drained by ONE strided f32->f16 copy (2 chunks of
800 cols at 1024-col stride), alternating Vector / Scalar engines so
both engines run in parallel and per-instruction overhead is amortized.
Output tiles [128, 3200] f16 go out as one 819 KB line-rate DMA per
pair block: 40 x 819.2 KB = 32.8 MB/core, ~92 us at the 358 GB/s
HBM-per-core limit, which is the design floor.

Shapes hardcoded from the problem spec:
  P=(100,100,100), Q=(4,4,8), R=(1,32,32,1), N=1<<20.
"""

import numpy as np

P0, P1, P2 = 100, 100, 100
Q0, Q1, Q2 = 4, 4, 8
R1, R2 = 32, 32
NCORES = 8

NPAIR = (P0 * P1) // NCORES       # 1250 pairs per core, exact
NPAIR_PAD = 1280                  # 40 blocks of 32 pairs
NPB = NPAIR_PAD // 32             # 40 pair-blocks
WCOLS = P2 * Q2                   # 800 output cols (i2, q2)
ACOLS = NPAIR_PAD * Q0            # 5120 abt cols (pair, q0)
ASPLIT = 1024                     # abt cols loaded in the first DMA (8 pbs)

_cache = {}


def _build_program():
    from concourse import bacc
    import concourse.mybir as mybir
    from concourse.tile import TileContext

    f32 = mybir.dt.float32
    f16 = mybir.dt.float16

    nc = bacc.Bacc("TRN2", target_bir_lowering=False, debug=False,
                   num_devices=NCORES)

    abt = nc.dram_tensor("abt", [128, ACOLS], f16, kind="ExternalInput")
    c2r = nc.dram_tensor("c2r", [128, WCOLS], f16, kind="ExternalInput")
    out = nc.dram_tensor("out", [NPB, 128, Q1, WCOLS], f16,
                         kind="ExternalOutput")

    with TileContext(nc) as tc:
        with tc.tile_pool(name="const", bufs=1) as cpool, \
             tc.tile_pool(name="ps", bufs=2, space="PSUM") as pspool, \
             tc.tile_pool(name="osb", bufs=4) as opool:

            # split the abt load so pair-blocks 0-7 can start while the
            # rest of the table is still in flight; c2r rides the SWDGE
            # (gpsimd) queue in parallel with the HWDGE loads.
            abt_a = cpool.tile([128, ASPLIT], f16, tag="abta")
            abt_b = cpool.tile([128, ACOLS - ASPLIT], f16, tag="abtb")
            c2_sb = cpool.tile([128, WCOLS], f16, tag="c2")
            nc.sync.dma_start(out=abt_a[:], in_=abt[:, 0:ASPLIT])
            nc.gpsimd.dma_start(out=c2_sb[:], in_=c2r[:])
            nc.sync.dma_start(out=abt_b[:], in_=abt[:, ASPLIT:])

            for pb in range(NPB):
                osb = opool.tile([128, Q1, WCOLS], f16, tag="osb")
                for half in range(2):
                    # one 4-bank PSUM tile holds two q1 outputs at a
                    # 1024-col (2-bank) stride so every matmul stays
                    # inside a single bank.
                    ps = pspool.tile([128, 2, 1024], f32, tag="ps")
                    for j in range(2):
                        q1 = 2 * half + j
                        if pb < ASPLIT // 128:
                            lhsT = abt_a[32 * q1:32 * (q1 + 1),
                                         pb * 128:(pb + 1) * 128]
                        else:
                            o = pb * 128 - ASPLIT
                            lhsT = abt_b[32 * q1:32 * (q1 + 1),
                                         o:o + 128]
                        nc.tensor.matmul(
                            ps[:, j, 0:512], lhsT,
                            c2_sb[32 * q1:32 * (q1 + 1), 0:512],
                            start=True, stop=True,
                            tile_position=(32 * q1, 0))
                        nc.tensor.matmul(
                            ps[:, j, 512:800], lhsT,
                            c2_sb[32 * q1:32 * (q1 + 1), 512:800],
                            start=True, stop=True,
                            tile_position=(32 * q1, 0))
                    # one strided drain copy per half: [128, 2, 800]
                    # (src stride 1024, dst contiguous), f32 -> f16.
                    eng = nc.vector.tensor_copy if half == 0 else \
                        nc.scalar.copy
                    eng(osb[:, 2 * half:2 * half + 2, :], ps[:, :, 0:800])
                nc.sync.dma_start(out=out[pb], in_=osb[:])
    nc.finalize()
    return nc


def _host_inputs(core0, core1, core2):
    """AB[p=(i0,i1), q0, (q1 r2)] on host (164 MFLOP), slice per core."""
    a0 = core0.reshape(P0 * Q0, R1)                       # [(i0 q0), r1]
    b = core1.reshape(P1, R1, Q1 * R2).transpose(1, 0, 2).reshape(
        R1, P1 * Q1 * R2)                                 # [r1, (i1 qr)]
    ab = (a0 @ b).reshape(P0, Q0, P1, Q1 * R2)            # [i0, q0, i1, qr]
    ab = ab.transpose(0, 2, 1, 3).reshape(P0 * P1, Q0, Q1 * R2)

    c2 = core2.reshape(P2, R2, Q2).transpose(1, 0, 2).reshape(R2, P2 * Q2)
    c2r = np.ascontiguousarray(np.tile(c2, (Q1, 1)), np.float16)

    in_maps = []
    for k in range(NCORES):
        abk = ab[k * NPAIR:(k + 1) * NPAIR]               # [1250, q0, qr]
        abk = abk.transpose(2, 0, 1).reshape(128, NPAIR * Q0)
        abt = np.zeros((128, ACOLS), np.float16)
        abt[:, :NPAIR * Q0] = abk
        in_maps.append({"abt": abt, "c2r": c2r})
    return in_maps


def run_device(core0, core1, core2, trace=False):
    from concourse.bass_utils import run_bass_kernel_spmd
    if "nc" not in _cache:
        _cache["nc"] = _build_program()
    nc = _cache["nc"]
    in_maps = _host_inputs(core0, core1, core2)
    res = run_bass_kernel_spmd(nc, in_maps, core_ids=list(range(NCORES)),
                               trace=trace)
    return res


def _reassemble(raw):
    # raw: [pb, (ps,q0), q1, (i2,q2)] -> value table [pair*100 + i2, 128]
    vt = raw.reshape(NPB, 32, Q0, Q1, P2, Q2)
    vt = vt.transpose(0, 1, 4, 2, 3, 5)          # [pb, ps, i2, q0, q1, q2]
    return np.ascontiguousarray(vt).reshape(NPAIR_PAD * P2, Q0 * Q1 * Q2)


def kernel(core0, core1, core2, indices):
    core0 = np.asarray(core0, np.float32)
    core1 = np.asarray(core1, np.float32)
    core2 = np.asarray(core2, np.float32)
    idx = np.asarray(indices)

    res = run_device(core0, core1, core2)
    vts = [_reassemble(r["out"]) for r in res.results]

    i2 = (idx % P2).astype(np.int64)
    i1 = ((idx // P2) % P1).astype(np.int64)
    i0 = (idx // (P1 * P2)).astype(np.int64)
    p = i0 * P1 + i1
    ck = p // NPAIR
    lp = p % NPAIR

    out = np.empty((idx.shape[0], Q0 * Q1 * Q2), np.float32)
    for k in range(NCORES):
        m = ck == k
        out[m] = vts[k][lp[m] * P2 + i2[m]]
    return out


# revision 6
# speedup vs baseline: 1.3447x; 1.0146x over previous
"""TT-embedding lookup kernel for 8 trn2 NeuronCores.

Strategy: the expanded embedding table (1e6 rows x 128 cols) is
materialized densely across the 8 cores -- core k owns the pair range
p = i0*100+i1 in [1250k, 1250(k+1)) -- and the host performs the final
index->row gather (the unshard step). The tiny first contraction
AB[pair] = a_{i0} @ b_{i1} (164 MFLOP total, ~0.4% of the FLOPs) is
folded into host-side input prep; the device runs the heavy stage:
for every block of 32 pairs,

  OUT[(pair,q0)=128, (i2,q2)=800] = ABt_q1.T @ C2_q1   per q1 (K=r2=32)

as fp16 matmuls (1 col/cycle on the PE vs 1/4 for f32) packed into
distinct PE row-groups (tile_position) and distinct PSUM banks. Each
4-bank PSUM tile is drained by ONE strided f32->f16 copy (2 chunks of
800 cols at 1024-col stride), alternating Vector / Scalar engines so
both engines run in parallel and per-instruction overhead is amortized.
Output tiles [128, 3200] f16 go out as one 819 KB line-rate DMA per
pair block: 40 x 819.2 KB = 32.8 MB/core, ~92 us at the 358 GB/s
HBM-per-core limit, which is the design floor.

Shapes hardcoded from the problem spec:
  P=(100,100,100), Q=(4,4,8), R=(1,32,32,1), N=1<<20.
"""

import numpy as np

P0, P1, P2 = 100, 100, 100
Q0, Q1, Q2 = 4, 4, 8
R1, R2 = 32, 32
NCORES = 8

NPAIR = (P0 * P1) // NCORES       # 1250 pairs per core, exact
NPAIR_PAD = 1280                  # 40 blocks of 32 pairs
NPB = NPAIR_PAD // 32             # 40 pair-blocks
LASTP = (NPAIR - 32 * (NPB - 1)) * Q0   # real partitions in last block: 8
WCOLS = P2 * Q2                   # 800 output cols (i2, q2)
ACOLS = NPAIR_PAD * Q0            # 5120 abt cols (pair, q0)
ASPLIT = 512                      # abt cols loaded in the first DMA (4 pbs)

_cache = {}


def _build_program():
    from concourse import bacc
    import concourse.mybir as mybir
    from concourse.tile import TileContext

    f32 = mybir.dt.float32
    f16 = mybir.dt.float16

    nc = bacc.Bacc("TRN2", target_bir_lowering=False, debug=False,
                   num_devices=NCORES)

    abt = nc.dram_tensor("abt", [128, ACOLS], f16, kind="ExternalInput")
    c2r = nc.dram_tensor("c2r", [128, WCOLS], f16, kind="ExternalInput")
    out = nc.dram_tensor("out", [NPB, 2, 128, 2, WCOLS], f16,
                         kind="ExternalOutput")

    with TileContext(nc) as tc:
        with tc.tile_pool(name="const", bufs=1) as cpool, \
             tc.tile_pool(name="ps", bufs=2, space="PSUM") as pspool, \
             tc.tile_pool(name="osb0", bufs=3) as opool0, \
             tc.tile_pool(name="osb1", bufs=3) as opool1:

            # split the abt load so pair-blocks 0-3 can start while the
            # rest of the table is still in flight; c2r rides the SWDGE
            # (gpsimd) queue in parallel with the HWDGE loads.
            abt_a = cpool.tile([128, ASPLIT], f16, tag="abta")
            abt_b = cpool.tile([128, ACOLS - ASPLIT], f16, tag="abtb")
            c2_sb = cpool.tile([128, WCOLS], f16, tag="c2")
            nc.sync.dma_start(out=abt_a[:], in_=abt[:, 0:ASPLIT])
            nc.gpsimd.dma_start(out=c2_sb[:], in_=c2r[:])
            nc.sync.dma_start(out=abt_b[:], in_=abt[:, ASPLIT:])

            for pb in range(NPB):
                np_ = LASTP if pb == NPB - 1 else 128
                for half in range(2):
                    # one 4-bank PSUM tile holds two q1 outputs at a
                    # 1024-col (2-bank) stride so every matmul stays
                    # inside a single bank.
                    ps = pspool.tile([128, 2, 1024], f32, tag="ps")
                    for j in range(2):
                        q1 = 2 * half + j
                        if pb < ASPLIT // 128:
                            lhsT = abt_a[32 * q1:32 * (q1 + 1),
                                         pb * 128:(pb + 1) * 128]
                        else:
                            o = pb * 128 - ASPLIT
                            lhsT = abt_b[32 * q1:32 * (q1 + 1),
                                         o:o + 128]
                        nc.tensor.matmul(
                            ps[:, j, 0:512], lhsT,
                            c2_sb[32 * q1:32 * (q1 + 1), 0:512],
                            start=True, stop=True,
                            tile_position=(32 * q1, 0))
                        nc.tensor.matmul(
                            ps[:, j, 512:800], lhsT,
                            c2_sb[32 * q1:32 * (q1 + 1), 512:800],
                            start=True, stop=True,
                            tile_position=(32 * q1, 0))
                    # Two independent drain->DMA streams: half 0 goes
                    # Vector copy -> sync-issued DMA; half 1 goes Scalar
                    # copy -> scalar-issued DMA (same-queue FIFO, no
                    # cross-engine wait). Each drain is ONE strided
                    # f32->f16 copy [128, 2, 800] (src stride 1024).
                    if half == 0:
                        osb = opool0.tile([128, 2, WCOLS], f16, tag="o0")
                        nc.vector.tensor_copy(osb[:], ps[:, :, 0:800])
                        nc.sync.dma_start(out=out[pb, 0, 0:np_],
                                          in_=osb[0:np_])
                    else:
                        osb = opool1.tile([128, 2, WCOLS], f16, tag="o1")
                        nc.scalar.copy(osb[:], ps[:, :, 0:800])
                        nc.scalar.dma_start(out=out[pb, 1, 0:np_],
                                            in_=osb[0:np_])
    nc.finalize()
    return nc


def _host_inputs(core0, core1, core2):
    """AB[p=(i0,i1), q0, (q1 r2)] on host (164 MFLOP), slice per core."""
    a0 = core0.reshape(P0 * Q0, R1)                       # [(i0 q0), r1]
    b = core1.reshape(P1, R1, Q1 * R2).transpose(1, 0, 2).reshape(
        R1, P1 * Q1 * R2)                                 # [r1, (i1 qr)]
    ab = (a0 @ b).reshape(P0, Q0, P1, Q1 * R2)            # [i0, q0, i1, qr]
    ab = ab.transpose(0, 2, 1, 3).reshape(P0 * P1, Q0, Q1 * R2)

    c2 = core2.reshape(P2, R2, Q2).transpose(1, 0, 2).reshape(R2, P2 * Q2)
    c2r = np.ascontiguousarray(np.tile(c2, (Q1, 1)), np.float16)

    in_maps = []
    for k in range(NCORES):
        abk = ab[k * NPAIR:(k + 1) * NPAIR]               # [1250, q0, qr]
        abk = abk.transpose(2, 0, 1).reshape(128, NPAIR * Q0)
        abt = np.zeros((128, ACOLS), np.float16)
        abt[:, :NPAIR * Q0] = abk
        in_maps.append({"abt": abt, "c2r": c2r})
    return in_maps


def run_device(core0, core1, core2, trace=False):
    from concourse.bass_utils import run_bass_kernel_spmd
    if "nc" not in _cache:
        _cache["nc"] = _build_program()
    nc = _cache["nc"]
    in_maps = _host_inputs(core0, core1, core2)
    res = run_bass_kernel_spmd(nc, in_maps, core_ids=list(range(NCORES)),
                               trace=trace)
    return res


def _reassemble(raw):
    # raw: [pb, half, (ps,q0), j, (i2,q2)] with q1 = 2*half + j
    # -> value table [pair*100 + i2, 128 = (q0, q1, q2)]
    vt = raw.reshape(NPB, 2, 32, Q0, 2, P2, Q2)
    vt = vt.transpose(0, 2, 5, 3, 1, 4, 6)   # [pb, ps, i2, q0, half, j, q2]
    return np.ascontiguousarray(vt).reshape(NPAIR_PAD * P2, Q0 * Q1 * Q2)


def kernel(core0, core1, core2, indices):
    core0 = np.asarray(core0, np.float32)
    core1 = np.asarray(core1, np.float32)
    core2 = np.asarray(core2, np.float32)
    idx = np.asarray(indices)

    res = run_device(core0, core1, core2)
    vts = [_reassemble(r["out"]) for r in res.results]

    i2 = (idx % P2).astype(np.int64)
    i1 = ((idx // P2) % P1).astype(np.int64)
    i0 = (idx // (P1 * P2)).astype(np.int64)
    p = i0 * P1 + i1
    ck = p // NPAIR
    lp = p % NPAIR

    out = np.empty((idx.shape[0], Q0 * Q1 * Q2), np.float32)
    for k in range(NCORES):
        m = ck == k
        out[m] = vts[k][lp[m] * P2 + i2[m]]
    return out


# revision 7
# speedup vs baseline: 1.5772x; 1.1729x over previous
"""TT-embedding lookup kernel for 8 trn2 NeuronCores.

Strategy: the expanded embedding table (1e6 rows x 128 cols) is
materialized densely across the 8 cores -- core k owns the pair range
p = i0*100+i1 in [1250k, 1250(k+1)) -- and the host performs the final
index->row gather (the unshard step). The tiny first contraction
AB[pair] = a_{i0} @ b_{i1} (164 MFLOP total, ~0.4% of the FLOPs) is
folded into host-side input prep; the device runs the heavy stage:
for every block of 32 pairs,

  OUT[(pair,q0)=128, (i2,q2)=800] = ABt_q1.T @ C2_q1   per q1 (K=r2=32)

as fp16 matmuls (1 col/cycle on the PE vs 1/4 for f32) packed into
distinct PE row-groups (tile_position) and distinct PSUM banks. Each
4-bank PSUM tile is drained by ONE strided f32->f16 copy (2 chunks of
800 cols at 1024-col stride), alternating Vector / Scalar engines so
both engines run in parallel and per-instruction overhead is amortized.
Output tiles [128, 3200] f16 go out as one 819 KB line-rate DMA per
pair block: 40 x 819.2 KB = 32.8 MB/core, ~92 us at the 358 GB/s
HBM-per-core limit, which is the design floor.

Shapes hardcoded from the problem spec:
  P=(100,100,100), Q=(4,4,8), R=(1,32,32,1), N=1<<20.
"""

import numpy as np

P0, P1, P2 = 100, 100, 100
Q0, Q1, Q2 = 4, 4, 8
R1, R2 = 32, 32
NCORES = 8

NPAIR = (P0 * P1) // NCORES       # 1250 pairs per core, exact
NPAIR_PAD = 1280                  # 40 blocks of 32 pairs
NPB = NPAIR_PAD // 32             # 40 pair-blocks
LASTP = (NPAIR - 32 * (NPB - 1)) * Q0   # real partitions in last block: 8
WCOLS = P2 * Q2                   # 800 output cols (i2, q2)
ACOLS = NPAIR_PAD * Q0            # 5120 abt cols (pair, q0)
ASPLIT = 512                      # abt cols loaded in the first DMA (4 pbs)

_cache = {}


def _build_program():
    from concourse import bacc
    import concourse.mybir as mybir
    from concourse.tile import TileContext

    f32 = mybir.dt.float32
    f16 = mybir.dt.float16

    nc = bacc.Bacc("TRN2", target_bir_lowering=False, debug=False,
                   num_devices=NCORES)

    abt = nc.dram_tensor("abt", [128, ACOLS], f16, kind="ExternalInput")
    c2r = nc.dram_tensor("c2r", [128, WCOLS], f16, kind="ExternalInput")
    out = nc.dram_tensor("out", [NPB, 2, 128, 2, WCOLS], f16,
                         kind="ExternalOutput")

    with TileContext(nc) as tc:
        with tc.tile_pool(name="const", bufs=1) as cpool, \
             tc.tile_pool(name="ps", bufs=4, space="PSUM") as pspool, \
             tc.tile_pool(name="osb0", bufs=3) as opool0, \
             tc.tile_pool(name="osb1", bufs=3) as opool1:

            # split the abt load so pair-blocks 0-3 can start early; the
            # bulk abt_b and c2r ride the SWDGE (gpsimd) queue so the
            # sync HWDGE ring carries nothing but output tiles.
            abt_a = cpool.tile([128, ASPLIT], f16, tag="abta")
            abt_b = cpool.tile([128, ACOLS - ASPLIT], f16, tag="abtb")
            c2_sb = cpool.tile([128, WCOLS], f16, tag="c2")
            nc.sync.dma_start(out=abt_a[:], in_=abt[:, 0:ASPLIT])
            nc.gpsimd.dma_start(out=c2_sb[:], in_=c2r[:])
            nc.gpsimd.dma_start(out=abt_b[:], in_=abt[:, ASPLIT:])

            for pb in range(NPB):
                np_ = LASTP if pb == NPB - 1 else 128
                osb0 = opool0.tile([128, 2, WCOLS], f16, tag="o0")
                osb1 = opool1.tile([128, 2, WCOLS], f16, tag="o1")
                for q1 in range(Q1):
                    # one 2-bank PSUM window per q1: short copy->matmul
                    # reuse chain (~2 us) that fits inside the 2.45 us
                    # per-block DMA period; 4 windows = all 8 banks.
                    ps = pspool.tile([128, 1024], f32, tag="ps")
                    if pb < ASPLIT // 128:
                        lhsT = abt_a[32 * q1:32 * (q1 + 1),
                                     pb * 128:(pb + 1) * 128]
                    else:
                        o = pb * 128 - ASPLIT
                        lhsT = abt_b[32 * q1:32 * (q1 + 1), o:o + 128]
                    nc.tensor.matmul(
                        ps[:, 0:512], lhsT,
                        c2_sb[32 * q1:32 * (q1 + 1), 0:512],
                        start=True, stop=True,
                        tile_position=(32 * q1, 0))
                    nc.tensor.matmul(
                        ps[:, 512:800], lhsT,
                        c2_sb[32 * q1:32 * (q1 + 1), 512:800],
                        start=True, stop=True,
                        tile_position=(32 * q1, 0))
                    # drain: one contiguous N=800 f32->f16 copy per
                    # window; q1 0,1 -> Vector, q1 2,3 -> Scalar.
                    osb, j = (osb0, q1) if q1 < 2 else (osb1, q1 - 2)
                    if q1 < 2:
                        nc.vector.tensor_copy(osb[:, j, :], ps[:, 0:800])
                    else:
                        nc.scalar.copy(osb[:, j, :], ps[:, 0:800])
                # both output DMAs issue from the otherwise-idle Sync
                # engine (HWDGE); streams stay independent via separate
                # osb pools.
                nc.sync.dma_start(out=out[pb, 0, 0:np_], in_=osb0[0:np_])
                nc.sync.dma_start(out=out[pb, 1, 0:np_], in_=osb1[0:np_])
    nc.finalize()
    return nc


def _host_inputs(core0, core1, core2):
    """AB[p=(i0,i1), q0, (q1 r2)] on host (164 MFLOP), slice per core."""
    a0 = core0.reshape(P0 * Q0, R1)                       # [(i0 q0), r1]
    b = core1.reshape(P1, R1, Q1 * R2).transpose(1, 0, 2).reshape(
        R1, P1 * Q1 * R2)                                 # [r1, (i1 qr)]
    ab = (a0 @ b).reshape(P0, Q0, P1, Q1 * R2)            # [i0, q0, i1, qr]
    ab = ab.transpose(0, 2, 1, 3).reshape(P0 * P1, Q0, Q1 * R2)

    c2 = core2.reshape(P2, R2, Q2).transpose(1, 0, 2).reshape(R2, P2 * Q2)
    c2r = np.ascontiguousarray(np.tile(c2, (Q1, 1)), np.float16)

    in_maps = []
    for k in range(NCORES):
        abk = ab[k * NPAIR:(k + 1) * NPAIR]               # [1250, q0, qr]
        abk = abk.transpose(2, 0, 1).reshape(128, NPAIR * Q0)
        abt = np.zeros((128, ACOLS), np.float16)
        abt[:, :NPAIR * Q0] = abk
        in_maps.append({"abt": abt, "c2r": c2r})
    return in_maps


def run_device(core0, core1, core2, trace=False):
    from concourse.bass_utils import run_bass_kernel_spmd
    if "nc" not in _cache:
        _cache["nc"] = _build_program()
    nc = _cache["nc"]
    in_maps = _host_inputs(core0, core1, core2)
    res = run_bass_kernel_spmd(nc, in_maps, core_ids=list(range(NCORES)),
                               trace=trace)
    return res


def _reassemble(raw):
    # raw: [pb, half, (ps,q0), j, (i2,q2)] with q1 = 2*half + j
    # -> value table [pair*100 + i2, 128 = (q0, q1, q2)]
    vt = raw.reshape(NPB, 2, 32, Q0, 2, P2, Q2)
    vt = vt.transpose(0, 2, 5, 3, 1, 4, 6)   # [pb, ps, i2, q0, half, j, q2]
    return np.ascontiguousarray(vt).reshape(NPAIR_PAD * P2, Q0 * Q1 * Q2)


def kernel(core0, core1, core2, indices):
    core0 = np.asarray(core0, np.float32)
    core1 = np.asarray(core1, np.float32)
    core2 = np.asarray(core2, np.float32)
    idx = np.asarray(indices)

    res = run_device(core0, core1, core2)
    vts = [_reassemble(r["out"]) for r in res.results]

    i2 = (idx % P2).astype(np.int64)
    i1 = ((idx // P2) % P1).astype(np.int64)
    i0 = (idx // (P1 * P2)).astype(np.int64)
    p = i0 * P1 + i1
    ck = p // NPAIR
    lp = p % NPAIR

    out = np.empty((idx.shape[0], Q0 * Q1 * Q2), np.float32)
    for k in range(NCORES):
        m = ck == k
        out[m] = vts[k][lp[m] * P2 + i2[m]]
    return out


# revision 10
# speedup vs baseline: 1.7098x; 1.0841x over previous
"""TT-embedding lookup kernel for 8 trn2 NeuronCores.

Strategy: the expanded embedding table (1e6 rows x 128 cols) is
materialized densely across the 8 cores -- core k owns the pair range
p = i0*100+i1 in [1250k, 1250(k+1)) -- and the host performs the final
index->row gather (the unshard step). The tiny first contraction
AB[pair] = a_{i0} @ b_{i1} (164 MFLOP total, ~0.4% of the FLOPs) is
folded into host-side input prep; the device runs the heavy stage:
for every block of 32 pairs,

  OUT[(pair,q0)=128, (i2,q2)=800] = ABt_q1.T @ C2_q1   per q1 (K=r2=32)

as fp16 matmuls (1 col/cycle on the PE vs 1/4 for f32) packed into
distinct PE row-groups (tile_position) and distinct PSUM banks. Each
4-bank PSUM tile is drained by ONE strided f32->f16 copy (2 chunks of
800 cols at 1024-col stride), alternating Vector / Scalar engines so
both engines run in parallel and per-instruction overhead is amortized.
Output tiles [128, 3200] f16 go out as one 819 KB line-rate DMA per
pair block: 40 x 819.2 KB = 32.8 MB/core, ~92 us at the 358 GB/s
HBM-per-core limit, which is the design floor.

Shapes hardcoded from the problem spec:
  P=(100,100,100), Q=(4,4,8), R=(1,32,32,1), N=1<<20.
"""

import numpy as np

P0, P1, P2 = 100, 100, 100
Q0, Q1, Q2 = 4, 4, 8
R1, R2 = 32, 32
NCORES = 8

NPAIR = (P0 * P1) // NCORES       # 1250 pairs per core, exact
NPAIR_PAD = 1280                  # 40 blocks of 32 pairs
NPB = NPAIR_PAD // 32             # 40 pair-blocks
LASTP = (NPAIR - 32 * (NPB - 1)) * Q0   # real partitions in last block: 8
WCOLS = P2 * Q2                   # 800 output cols (i2, q2)
ACOLS = NPAIR_PAD * Q0            # 5120 abt cols (pair, q0)
ASPLIT = 128                      # abt cols for pair-block 0 (sync ring)
BSPLIT = 1152                     # abt cols up to pair-block 8 (scalar ring)

_cache = {}


def _build_program():
    from concourse import bacc
    import concourse.mybir as mybir
    from concourse.tile import TileContext

    f32 = mybir.dt.float32
    f16 = mybir.dt.float16

    nc = bacc.Bacc("TRN2", target_bir_lowering=False, debug=False,
                   num_devices=NCORES)

    abt = nc.dram_tensor("abt", [128, ACOLS], f16, kind="ExternalInput")
    c2r = nc.dram_tensor("c2r", [128, WCOLS], f16, kind="ExternalInput")
    out = nc.dram_tensor("out", [NPB, 2, 128, 2, WCOLS], f16,
                         kind="ExternalOutput")

    with TileContext(nc) as tc:
        with tc.tile_pool(name="const", bufs=1) as cpool, \
             tc.tile_pool(name="ps", bufs=4, space="PSUM") as pspool, \
             tc.tile_pool(name="osb0", bufs=3) as opool0, \
             tc.tile_pool(name="osb1", bufs=3) as opool1:

            # input loads spread over all three DMA rings so pair-block 0
            # can start ~9 us earlier: pb0's weights + c2r on sync, the
            # next 8 blocks' weights on the scalar HWDGE ring, the bulk
            # on the SWDGE (gpsimd) ring.
            abt_a = cpool.tile([128, ASPLIT], f16, tag="abta")
            abt_b = cpool.tile([128, BSPLIT - ASPLIT], f16, tag="abtb")
            abt_c = cpool.tile([128, ACOLS - BSPLIT], f16, tag="abtc")
            c2_sb = cpool.tile([128, WCOLS], f16, tag="c2")
            nc.sync.dma_start(out=abt_a[:], in_=abt[:, 0:ASPLIT])
            nc.scalar.dma_start(out=abt_b[:], in_=abt[:, ASPLIT:BSPLIT])
            nc.gpsimd.dma_start(out=abt_c[:], in_=abt[:, BSPLIT:])
            nc.sync.dma_start(out=c2_sb[:], in_=c2r[:])

            for pb in range(NPB):
                np_ = LASTP if pb == NPB - 1 else 128
                osb0 = opool0.tile([128, 2, WCOLS], f16, tag="o0")
                osb1 = opool1.tile([128, 2, WCOLS], f16, tag="o1")
                for q1 in range(Q1):
                    # one 2-bank PSUM window per q1: short copy->matmul
                    # reuse chain (~2 us) that fits inside the 2.45 us
                    # per-block DMA period; 4 windows = all 8 banks.
                    ps = pspool.tile([128, 1024], f32, tag="ps")
                    if pb < ASPLIT // 128:
                        lhsT = abt_a[32 * q1:32 * (q1 + 1),
                                     pb * 128:(pb + 1) * 128]
                    elif pb < BSPLIT // 128:
                        o = pb * 128 - ASPLIT
                        lhsT = abt_b[32 * q1:32 * (q1 + 1), o:o + 128]
                    else:
                        o = pb * 128 - BSPLIT
                        lhsT = abt_c[32 * q1:32 * (q1 + 1), o:o + 128]
                    nc.tensor.matmul(
                        ps[:, 0:512], lhsT,
                        c2_sb[32 * q1:32 * (q1 + 1), 0:512],
                        start=True, stop=True,
                        tile_position=(32 * q1, 0))
                    nc.tensor.matmul(
                        ps[:, 512:800], lhsT,
                        c2_sb[32 * q1:32 * (q1 + 1), 512:800],
                        start=True, stop=True,
                        tile_position=(32 * q1, 0))
                    # drain: one contiguous N=800 f32->f16 copy per
                    # window; q1 0,1 -> Vector, q1 2,3 -> Scalar.
                    osb, j = (osb0, q1) if q1 < 2 else (osb1, q1 - 2)
                    if q1 < 2:
                        nc.vector.tensor_copy(osb[:, j, :], ps[:, 0:800])
                    else:
                        nc.scalar.copy(osb[:, j, :], ps[:, 0:800])
                # both output DMAs issue from the otherwise-idle Sync
                # engine (HWDGE); streams stay independent via separate
                # osb pools.
                nc.sync.dma_start(out=out[pb, 0, 0:np_], in_=osb0[0:np_])
                nc.sync.dma_start(out=out[pb, 1, 0:np_], in_=osb1[0:np_])
    nc.finalize()
    return nc


def _host_inputs(core0, core1, core2):
    """AB[p=(i0,i1), q0, (q1 r2)] on host (164 MFLOP), slice per core."""
    a0 = core0.reshape(P0 * Q0, R1)                       # [(i0 q0), r1]
    b = core1.reshape(P1, R1, Q1 * R2).transpose(1, 0, 2).reshape(
        R1, P1 * Q1 * R2)                                 # [r1, (i1 qr)]
    ab = (a0 @ b).reshape(P0, Q0, P1, Q1 * R2)            # [i0, q0, i1, qr]
    ab = ab.transpose(0, 2, 1, 3).reshape(P0 * P1, Q0, Q1 * R2)

    c2 = core2.reshape(P2, R2, Q2).transpose(1, 0, 2).reshape(R2, P2 * Q2)
    c2r = np.ascontiguousarray(np.tile(c2, (Q1, 1)), np.float16)

    in_maps = []
    for k in range(NCORES):
        abk = ab[k * NPAIR:(k + 1) * NPAIR]               # [1250, q0, qr]
        abk = abk.transpose(2, 0, 1).reshape(128, NPAIR * Q0)
        abt = np.zeros((128, ACOLS), np.float16)
        abt[:, :NPAIR * Q0] = abk
        in_maps.append({"abt": abt, "c2r": c2r})
    return in_maps


def run_device(core0, core1, core2, trace=False):
    from concourse.bass_utils import run_bass_kernel_spmd
    if "nc" not in _cache:
        _cache["nc"] = _build_program()
    nc = _cache["nc"]
    in_maps = _host_inputs(core0, core1, core2)
    res = run_bass_kernel_spmd(nc, in_maps, core_ids=list(range(NCORES)),
                               trace=trace)
    return res


def _reassemble(raw):
    # raw: [pb, half, (ps,q0), j, (i2,q2)] with q1 = 2*half + j
    # -> value table [pair*100 + i2, 128 = (q0, q1, q2)]
    vt = raw.reshape(NPB, 2, 32, Q0, 2, P2, Q2)
    vt = vt.transpose(0, 2, 5, 3, 1, 4, 6)   # [pb, ps, i2, q0, half, j, q2]
    return np.ascontiguousarray(vt).reshape(NPAIR_PAD * P2, Q0 * Q1 * Q2)


def kernel(core0, core1, core2, indices):
    core0 = np.asarray(core0, np.float32)
    core1 = np.asarray(core1, np.float32)
    core2 = np.asarray(core2, np.float32)
    idx = np.asarray(indices)

    res = run_device(core0, core1, core2)
    vts = [_reassemble(r["out"]) for r in res.results]

    i2 = (idx % P2).astype(np.int64)
    i1 = ((idx // P2) % P1).astype(np.int64)
    i0 = (idx // (P1 * P2)).astype(np.int64)
    p = i0 * P1 + i1
    ck = p // NPAIR
    lp = p % NPAIR

    out = np.empty((idx.shape[0], Q0 * Q1 * Q2), np.float32)
    for k in range(NCORES):
        m = ck == k
        out[m] = vts[k][lp[m] * P2 + i2[m]]
    return out
